# revision 68
# baseline (speedup 1.0000x reference)
"""HardAttention Bass kernel for 8 TRN2 NeuronCores.

reference math (B=32, T=4096, H=256):
  energy[b,t,h] = relu( sum_k cat(hidden,enc)[b,t,k] * attn_w[h,k] + attn_b[h] )
  scores[b,t]   = sum_h energy[b,t,h] * v[h]
  out           = softmax(scores, axis=t)[:, None, :]

Device strategy (data-parallel over B, 4 batches/core):
  * host folds: W2v = W2 * v, qv = (hidden @ W1.T + b) * v  (valid since
    v >= 0: relu(x)*v == relu(x*v)); enc ships as fp16 [b, k, t] tiles
    (k on SBUF partitions), halving HBM traffic vs fp32
  * per (b, 512-col sub): z[h,t] = W2v-tiles @ enc-tiles on PE (fp16
    matmuls at full rate, fp32 psum); relu+bias fused per h-chunk on
    ACT (activation bias=) / DVE (tensor_scalar add+max), writing fp16
    r tiles; DVE folds the two h-chunks per 1024-col pair (fp16 2x add)
  * h-reduction (128 partitions -> scores): batches b0..b2 on the Pool
    engine (gpsimd C-axis tensor_reduce into a partition-0 strip,
    gathered to [12,1024] via a DRAM bounce); the last batch on PE via
    indicator matmuls into a [8,512] psum tile -- PE is idle by then and
    this keeps the drain off Pool's serial queue
  * softmax split: b0..b2 exp+accum / per-batch totals (block-ones
    matmul) / reciprocal / scale / DMA all hide under b3 compute; b3's
    short chain (exp8 -> gs8 -> recip -> scale -> DMA) is the only tail
  * DMA: one fp16 stream on the SP queue (first tiles split small so PE
    starts at ~3.7us), consts packed into two descriptors
Host reassembles [32,512] rows -> [4, 4096] per core (pure reshape).
"""

from contextlib import ExitStack

import numpy as np

import concourse.bass as bass
import concourse.tile as tile
from concourse import bacc, mybir
from concourse.bass_utils import run_bass_kernel_spmd

B, T, H = 32, 4096, 256
NCORES = 8
BC = B // NCORES            # 4 batches per core
KC = H // 128               # 2 k-chunks
HC = H // 128               # 2 h-chunks
SUB = 512                   # t columns per reduction unit
NSUB = T // SUB             # 8 subs per batch
DMAW = 2048                 # t columns per enc DMA
NDMA = T // DMAW            # 2 DMA halves per (b, kc)

F32 = mybir.dt.float32
F16 = mybir.dt.float16

_CACHE = {}
LAST_RESULTS = None


def _build():
    if "nc" in _CACHE:
        return _CACHE["nc"]

    nc = bacc.Bacc(None, target_bir_lowering=False)
    enc_d = nc.dram_tensor("enc", [BC, KC, 128, T], F16, kind="ExternalInput")
    # c16: cols 0:512 = w2v lhsT tiles; cols 512:768 = 16 blocks of
    # [128,16] (column j of block j all-ones) for the b3 PE reductions
    c16_d = nc.dram_tensor("c16", [128, 512 + 256], F16, kind="ExternalInput")
    # c32: cols 0:8 = qv (per-(b,hc) bias columns); cols 8:20 rows 0:12 =
    # block-ones(4) over b0..b2 rows; cols 20:36 rows 0:16 = all-ones 16x16
    c32_d = nc.dram_tensor("c32", [128, 40], F32, kind="ExternalInput")
    out_d = nc.dram_tensor("scores", [32, SUB], F32, kind="ExternalOutput")
    sc_d = nc.dram_tensor("sc_scratch", [12, 1024], F32, kind="Internal")

    AF = mybir.ActivationFunctionType
    ALU = mybir.AluOpType

    NP = NSUB // 2          # 4 sub-pairs per batch
    PW = 2 * SUB            # 1024 columns per pair

    # engine for each hc1 relu by (b, sub); hc0 relus always run on ACT
    HC1 = {}
    for b in range(BC):
        for s in range(NSUB):
            HC1[(b, s)] = "dve"
    for b in range(BC):
        for s in [0, 4]:
            HC1[(b, s)] = "act"

    with tile.TileContext(nc) as tc, ExitStack() as ctx:
        const = ctx.enter_context(tc.tile_pool(name="const", bufs=1))
        encp = ctx.enter_context(tc.tile_pool(name="encp", bufs=1))
        zp = ctx.enter_context(tc.tile_pool(name="zp", bufs=6, space="PSUM"))
        rsp = ctx.enter_context(tc.tile_pool(name="rsp", bufs=6))
        b3r = ctx.enter_context(tc.tile_pool(name="b3r", bufs=1))
        tailp = ctx.enter_context(tc.tile_pool(name="tail", bufs=1))
        pscp = ctx.enter_context(tc.tile_pool(name="pscp", bufs=1, space="PSUM"))

        # fp16 consts in one DMA on the SP queue ahead of the enc stream;
        # fp32 consts follow the first small enc tiles
        c16_sb = const.tile([128, 512 + 256], F16, tag="c16")
        nc.sync.dma_start(c16_sb[:], c16_d[:])
        c32_sb = const.tile([128, 40], F32, tag="c32")
        w16_sb = c16_sb
        qv_sb = c32_sb


        def w2v_ap(kc, hc):
            off = (kc * HC + hc) * 128
            return w16_sb[:, off : off + 128]

        def ind_ap(j):
            return c16_sb[:, 512 + 16 * j : 512 + 16 * j + 16]


        # stream in all enc tiles (resident: 8.4 MB total). The first two
        # (b0, half0) tiles are split [0:512]+[512:2048] so PE's first pair
        # only waits on two small transfers.
        enc_t = {}
        first = {}
        for kc in range(KC):
            ft = encp.tile([128, SUB], F16, tag=f"enc_f_{kc}")
            nc.gpsimd.dma_start(ft[:], enc_d[0, kc][:, 0:SUB])
            first[kc] = ft
        nc.sync.dma_start(c32_sb[:], c32_d[:])

        HW = DMAW // 2
        for b in range(BC):
            for half in range(NDMA):
                for kc in range(KC):
                    et = encp.tile([128, DMAW], F16, tag=f"enc_{b}_{kc}_{half}")
                    enc_t[(b, kc, half)] = et
                if b == 0:
                    # finer chunks for the first batch keep PE fed while it
                    # chases the incoming stream
                    for kc in range(KC):
                        for q in range(2):
                            lo = half * DMAW + q * HW
                            nc.sync.dma_start(
                                enc_t[(b, kc, half)][:, q * HW : (q + 1) * HW],
                                enc_d[b, kc][:, lo : lo + HW],
                            )
                else:
                    for kc in range(KC):
                        nc.sync.dma_start(
                            enc_t[(b, kc, half)][:],
                            enc_d[b, kc][:, half * DMAW : (half + 1) * DMAW],
                        )

        scores12 = tailp.tile([12, PW], F32, tag="scores12")
        sall = tailp.tile([1, 12 * PW], F32, tag="sall")
        exp12t = tailp.tile([12, PW], F32, tag="exp12t")
        sums12 = tailp.tile([12, 1], F32, tag="sums12")
        psc16 = pscp.tile([16, 256], F32, tag="psc16")


        r01_b3 = []
        _rows_done = set()

        def do_pair(b, p):
            r0p = rsp.tile([128, PW], F16, tag="r0p")
            r1p = rsp.tile([128, PW], F16, tag="r1p")

            def dve_relu(dst, zsrc, hc):
                nc.vector.tensor_scalar(
                    dst, zsrc,
                    scalar1=qv_sb[:, b * HC + hc : b * HC + hc + 1],
                    scalar2=0.0,
                    op0=ALU.add,
                    op1=ALU.max,
                )

            for s in range(2):
                sub = 2 * p + s
                half, col = divmod(sub * SUB, DMAW)
                zs = []
                for hc in range(HC):
                    z = zp.tile([128, SUB], F32, tag="z")
                    for kc in range(KC):
                        if b == 0 and sub == 0:
                            rhs = first[kc][:]
                        else:
                            rhs = enc_t[(b, kc, half)][:, col : col + SUB]
                        nc.tensor.matmul(
                            z[:],
                            w2v_ap(kc, hc),
                            rhs,
                            start=(kc == 0),
                            stop=(kc == KC - 1),
                        )
                    zs.append(z)
                cols = slice(s * SUB, (s + 1) * SUB)
                nc.scalar.activation(
                    r0p[:, cols], zs[0][:], AF.Relu,
                    bias=qv_sb[:, b * HC : b * HC + 1],
                )
                eng = HC1.get((b, sub), "dve")
                if eng == "act":
                    nc.scalar.activation(
                        r1p[:, cols], zs[1][:], AF.Relu,
                        bias=qv_sb[:, b * HC + 1 : b * HC + 2],
                    )
                elif eng == "pool":
                    nc.gpsimd.tensor_scalar(
                        r1p[:, cols], zs[1][:],
                        scalar1=qv_sb[:, b * HC + 1 : b * HC + 2],
                        scalar2=0.0,
                        op0=ALU.add,
                        op1=ALU.max,
                    )
                else:
                    dve_relu(r1p[:, cols], zs[1][:], 1)
            if b == BC - 1:
                r01p = b3r.tile([128, PW], F16, tag=f"r01b3_{p}")
            else:
                r01p = rsp.tile([128, PW], F16, tag="r01p")
            nc.vector.tensor_tensor(r01p[:], r0p[:], r1p[:], op=ALU.add)
            if b < BC - 1:
                row = b * NP + p
                nc.gpsimd.tensor_reduce(
                    sall[:, row * PW : (row + 1) * PW], r01p[:],
                    axis=mybir.AxisListType.C, op=ALU.add,
                )
                if row == 7:
                    nc.sync.dma_start(sc_d[0:8], sall[:, 0 : 8 * PW])
                    nc.sync.dma_start(scores12[0:8], sc_d[0:8])
            else:
                r01_b3.append(r01p)

        def b3_red(p):
            # psc16 row 4p+2s+h <- sum over partitions of pair p's quarter
            # (s, h); row order is t-linear for the final byte-linear DMA
            for s in range(2):
                for h in range(2):
                    j = 4 * p + 2 * s + h
                    lo = s * SUB + h * 256
                    nc.tensor.matmul(
                        psc16[:], ind_ap(j),
                        r01_b3[p][:, lo : lo + 256],
                        start=(p == 0 and s == 0 and h == 0),
                        stop=(p == NP - 1 and s == 1 and h == 1),
                    )

        for b in range(BC - 1):
            for p in range(NP):
                do_pair(b, p)

        # b3 main compute; the b0..b2 tail ops are emitted at stream
        # positions where their deps are already satisfied, and the b3
        # reduction matmuls lag one pair behind the fold that feeds them
        do_pair(BC - 1, 0)
        # gather the rest of the partition-0 scores strip (rows 8-11)
        # via the DRAM bounce (rows 0-7 were bounced right after b1)
        nc.sync.dma_start(sc_d[8:12], sall[:, 8 * PW : 12 * PW])
        nc.sync.dma_start(scores12[8:12], sc_d[8:12])
        # exp for b0..b2 (ACT stream: lands after b3p0's relus)
        nc.scalar.activation(
            exp12t[:], scores12[:], AF.Exp, accum_out=sums12[:],
        )
        do_pair(BC - 1, 1)
        b3_red(0)
        do_pair(BC - 1, 2)
        b3_red(1)
        # b0..b2 totals: the tiny matmul goes here so PE reaches it long
        # before its last main matmul (exp12's accum is ready by now), and
        # the whole 12-row tail (recip/scale/DMA) hides under b3 compute
        gs12 = pscp.tile([12, 1], F32, tag="gsx")
        nc.tensor.matmul(gs12[:], c32_sb[0:12, 8:20], sums12[:],
                         start=True, stop=True)
        recip12 = tailp.tile([12, 1], F32, tag="recip12")
        nc.vector.reciprocal(recip12[:], gs12[:])
        outs12 = tailp.tile([12, PW], F32, tag="outs12")
        nc.vector.tensor_scalar_mul(outs12[:], exp12t[:], recip12[:])
        # SWDGE: descriptor gen on the idle Pool engine, keeping the shared
        # HWDGE generator free for the critical final DMA
        nc.gpsimd.dma_start(out_d[0:24], outs12[:])
        do_pair(BC - 1, 3)
        b3_red(2)
        b3_red(3)

        # ---- remaining tail ----
        exp16b = tailp.tile([16, 256], F32, tag="exp16b")
        acc16 = tailp.tile([16, 1], F32, tag="acc16")
        nc.scalar.activation(exp16b[:], psc16[:], AF.Exp, accum_out=acc16[:])
        gs16 = pscp.tile([16, 1], F32, tag="gsx")
        nc.tensor.matmul(gs16[:], c32_sb[0:16, 20:36], acc16[:],
                         start=True, stop=True)
        recip16b = tailp.tile([16, 1], F32, tag="recip16b")
        nc.vector.reciprocal(recip16b[:], gs16[:])
        outs16b = tailp.tile([16, 256], F32, tag="outs16b")
        nc.vector.tensor_scalar_mul(outs16b[:], exp16b[:], recip16b[:])
        nc.sync.dma_start(out_d[24:32], outs16b[:])

    nc.compile()
    _CACHE["nc"] = nc
    return nc


def _prep_inputs(hidden, encoder_outputs, attn_w, attn_b, v):
    w1 = attn_w[:, :H]
    w2 = attn_w[:, H:]
    qv_full = (((hidden @ w1.T) + attn_b) * v).astype(np.float32)   # [B, H]
    w2v = (w2 * v[:, None]).astype(np.float32)     # [H(h), H(k)]
    w2v_T = np.ascontiguousarray(w2v.T)            # [k, h]

    w16 = np.zeros((128, KC * HC * 128), dtype=np.float16)
    for kc in range(KC):
        for hc in range(HC):
            off = (kc * HC + hc) * 128
            w16[:, off : off + 128] = w2v_T[
                kc * 128 : (kc + 1) * 128, hc * 128 : (hc + 1) * 128
            ].astype(np.float16)

    c16 = np.zeros((128, 512 + 256), dtype=np.float16)
    c16[:, 0:512] = w16
    for j in range(16):
        c16[:, 512 + 16 * j + j] = 1.0

    enc16 = encoder_outputs.astype(np.float16)     # [T, B, H]
    in_maps = []
    for c in range(NCORES):
        bs = c * BC
        enc_c = np.ascontiguousarray(
            enc16[:, bs : bs + BC, :].transpose(1, 2, 0)
        ).reshape(BC, KC, 128, T)
        qv_c = np.ascontiguousarray(
            qv_full[bs : bs + BC].reshape(BC, HC, 128).transpose(2, 0, 1)
        ).reshape(128, BC * HC)
        c32 = np.zeros((128, 40), dtype=np.float32)
        c32[:, 0:8] = qv_c
        for bb in range(3):
            c32[bb * 4 : (bb + 1) * 4, 8 + bb * 4 : 8 + (bb + 1) * 4] = 1.0
        c32[0:16, 20:36] = 1.0
        in_maps.append(
            {"enc": enc_c, "c16": c16, "c32": c32}
        )
    return in_maps


def kernel(hidden, encoder_outputs, attn_w, attn_b, v):
    global LAST_RESULTS
    nc = _build()
    in_maps = _prep_inputs(
        np.asarray(hidden, dtype=np.float32),
        np.asarray(encoder_outputs, dtype=np.float32),
        np.asarray(attn_w, dtype=np.float32),
        np.asarray(attn_b, dtype=np.float32),
        np.asarray(v, dtype=np.float32),
    )
    res = run_bass_kernel_spmd(nc, in_maps, list(range(NCORES)))
    LAST_RESULTS = res
    out = np.empty((B, 1, T), dtype=np.float32)
    for c in range(NCORES):
        out[c * BC : (c + 1) * BC, 0, :] = (
            res.results[c]["scores"].reshape(BC, T)
        )
    return out
